# revision 9
# baseline (speedup 1.0000x reference)
"""Trainium2 Bass kernel for nn_EnhancedMultiHeadAttention (multi-scale MHA + gating + LN).

Sharding: 8 cores = 4 batches x 2 T-halves. Each core computes the full
attention stack for its (batch, query-half) on all 16 heads and 3 scales.
The query-half selection is made STATIC across the SPMD program by rolling
the time axis of x on the host for odd cores (attention is permutation-
invariant over key positions; avg-pool seam errors are fixed by small
host-computed correction vectors applied to 8 fixed columns).

All matmuls run in bf16 (fp32 PSUM accumulation). Softmax skips the max
subtraction (|logits| < 1 for these inputs by construction; verified).
"""
import numpy as np
import ml_dtypes

import concourse.bacc as bacc
import concourse.bass as bass
import concourse.mybir as mybir
from concourse import tile
from concourse.bass_utils import run_bass_kernel_spmd

F32 = mybir.dt.float32
BF16 = mybir.dt.bfloat16
AF = mybir.ActivationFunctionType

B, T, D = 4, 1024, 1024
H, DK = 16, 64
S = 3
SCALES = (1, 3, 5)
TQ = 512            # queries per core
C = D // 128        # 8 chunks of the model dim
N_CORES = 8
LN_EPS = 1e-5
# columns (rolled T index) that may need pooling seam corrections
FIX_COLS = (0, 1, 510, 511, 512, 513, 1022, 1023)

_cache = {}


def _build():
    nc = bacc.Bacc("TRN2", target_bir_lowering=False, debug=False,
                   enable_asserts=True, num_devices=N_CORES)

    xt = nc.dram_tensor("xt", [D, T], F32, kind="ExternalInput").ap()
    wqk = nc.dram_tensor("wqk", [S, D, 2 * D], BF16, kind="ExternalInput").ap()
    wv = nc.dram_tensor("wv", [S, D, D], BF16, kind="ExternalInput").ap()
    bq = nc.dram_tensor("bq", [S, D], F32, kind="ExternalInput").ap()
    wo = nc.dram_tensor("wo", [D, D], BF16, kind="ExternalInput").ap()
    ob = nc.dram_tensor("ob", [S, D], F32, kind="ExternalInput").ap()
    gw = nc.dram_tensor("gw", [S, D, S], BF16, kind="ExternalInput").ap()
    gb = nc.dram_tensor("gb", [S], F32, kind="ExternalInput").ap()
    lng = nc.dram_tensor("lng", [D], F32, kind="ExternalInput").ap()
    lnb = nc.dram_tensor("lnb", [D], F32, kind="ExternalInput").ap()
    ident = nc.dram_tensor("ident", [128, 128], BF16, kind="ExternalInput").ap()
    corr = nc.dram_tensor("corr", [2, 8, C, 128], F32, kind="ExternalInput").ap()
    yt = nc.dram_tensor("yt", [D, TQ], F32, kind="ExternalOutput").ap()

    with tile.TileContext(nc) as tc:
        _emit(tc, xt=xt, wqk=wqk, wv=wv, bq=bq, wo=wo, ob=ob, gw=gw, gb=gb,
              lng=lng, lnb=lnb, ident=ident, corr=corr, yt=yt)
    nc.compile()
    return nc


def _emit(tc, *, xt, wqk, wv, bq, wo, ob, gw, gb, lng, lnb, ident, corr, yt):
    nc = tc.nc
    pers_ctx = tc.tile_pool(name="pers", bufs=1)
    pers = pers_ctx.__enter__()

    # ---- persistent SBUF tensors -------------------------------------------
    xT = pers.tile([128, C, T], F32, name="xT")            # x^T  [D, T]
    A = pers.tile([128, C, T], BF16, name="A")             # pooled (unscaled) x^T
    qT = pers.tile([128, C, TQ], BF16, name="qT")          # q^T  [D, TQ]
    kT = pers.tile([128, C, T], BF16, name="kT")           # k^T  [D, T]
    v_sb = pers.tile([128, C, H, DK + 1], BF16, name="v_sb")   # v [T, D] (+ones col)
    o_sb = pers.tile([128, TQ // 128, D], BF16, name="o_sb")   # o  [TQ, D]
    oT_sb = pers.tile([128, C, TQ], BF16, name="oT_sb")    # o^T [D, TQ]
    outs = [pers.tile([128, C, TQ], BF16, name=f"out{i}") for i in range(S)]
    h_sb = pers.tile([128, C, TQ], F32, name="h_sb")       # residual+combined [D, TQ]

    ident_sb = pers.tile([128, 128], BF16, name="ident_sb")
    nc.sync.dma_start(out=ident_sb[:], in_=ident[:])
    bq_sb = pers.tile([128, S, C], F32, name="bq_sb")
    nc.sync.dma_start(out=bq_sb[:], in_=bq.rearrange("s (c p) -> p s c", p=128))
    ob_sb = pers.tile([128, S, C], F32, name="ob_sb")
    nc.sync.dma_start(out=ob_sb[:], in_=ob.rearrange("s (c p) -> p s c", p=128))
    gw_sb = pers.tile([128, S, C, S], BF16, name="gw_sb")
    nc.sync.dma_start(out=gw_sb[:], in_=gw.rearrange("s (c p) t -> p s c t", p=128))
    gb_sb = pers.tile([S, 1], F32, name="gb_sb")
    nc.sync.dma_start(out=gb_sb[:], in_=gb[:, None])
    lng_sb = pers.tile([128, C], F32, name="lng_sb")
    nc.sync.dma_start(out=lng_sb[:], in_=lng.rearrange("(c p) -> p c", p=128))
    lnb_sb = pers.tile([128, C], F32, name="lnb_sb")
    nc.sync.dma_start(out=lnb_sb[:], in_=lnb.rearrange("(c p) -> p c", p=128))
    corr_sb = pers.tile([128, 2, 8, C], F32, name="corr_sb")
    nc.sync.dma_start(out=corr_sb[:], in_=corr.rearrange("s k c p -> p s k c", p=128))

    nc.sync.dma_start(out=xT[:], in_=xt.rearrange("(c p) t -> p c t", p=128))
    nc.vector.memset(v_sb[:, :, :, DK:DK + 1], 1.0)      # ones col for denominator
    ones_sb = pers.tile([128, 1], BF16, name="ones_sb")
    nc.vector.memset(ones_sb[:], 1.0)

    with tc.tile_pool(name="wpool", bufs=2) as wpool, \
         tc.tile_pool(name="cpool", bufs=2) as cpool, \
         tc.tile_pool(name="ps", bufs=2, space="PSUM") as ps:

        for si, scale in enumerate(SCALES):
            # ---- pooling: A = unscaled window sum of xT along T ------------
            if scale == 3:
                nc.vector.tensor_add(A[:, :, 1:T], xT[:, :, 1:T], xT[:, :, 0:T - 1])
                nc.vector.tensor_copy(A[:, :, 0:1], xT[:, :, 0:1])
                nc.vector.tensor_add(A[:, :, 0:T - 1], A[:, :, 0:T - 1], xT[:, :, 1:T])
            elif scale == 5:
                nc.vector.tensor_add(A[:, :, 2:T], A[:, :, 2:T], xT[:, :, 0:T - 2])
                nc.vector.tensor_add(A[:, :, 0:T - 2], A[:, :, 0:T - 2], xT[:, :, 2:T])
            if scale != 1:
                for k in range(8):
                    col = FIX_COLS[k]
                    nc.vector.tensor_add(A[:, :, col], A[:, :, col],
                                         corr_sb[:, si - 1, k, :])
            src = xT if scale == 1 else A
            if scale == 1:
                # matmuls need bf16; make a bf16 copy of xT into A
                for c in range(C):
                    nc.vector.tensor_copy(A[:, c, :], xT[:, c, :])
                src = A

            # ---- q^T / k^T projections ------------------------------------
            # q^T rows 0:1024 = wqk cols 0:1024; k^T = wqk cols 1024:2048
            for rc in range(C):          # q row-chunks
                mm = ps.tile([128, TQ], F32, tag="mm", bufs=2, name="mm_q")
                for dc in range(C):
                    wt = wpool.tile([128, 128], BF16, tag="wqk", bufs=10, name="wt_q")
                    nc.sync.dma_start(out=wt[:], in_=wqk[si, dc * 128:(dc + 1) * 128,
                                                        rc * 128:(rc + 1) * 128])
                    nc.tensor.matmul(mm[:], wt[:], src[:, dc, 0:TQ],
                                     start=(dc == 0), stop=(dc == C - 1))
                nc.scalar.activation(qT[:, rc, :], mm[:], AF.Identity,
                                     bias=bq_sb[:, si, rc:rc + 1], scale=1.0)
            for rc in range(C):          # k row-chunks
                for th in range(2):
                    mm = ps.tile([128, TQ], F32, tag="mm", bufs=2, name="mm_k")
                    for dc in range(C):
                        wt = wpool.tile([128, 128], BF16, tag="wqk", bufs=10, name="wt_k")
                        nc.sync.dma_start(out=wt[:], in_=wqk[si, dc * 128:(dc + 1) * 128,
                                                            D + rc * 128:D + (rc + 1) * 128])
                        nc.tensor.matmul(mm[:], wt[:], src[:, dc, th * TQ:(th + 1) * TQ],
                                         start=(dc == 0), stop=(dc == C - 1))
                    nc.vector.tensor_copy(kT[:, rc, th * TQ:(th + 1) * TQ], mm[:])

            # ---- v^T projection + transpose to v [T, D] --------------------
            for rc in range(C):          # v^T row-chunks (v columns)
                for th in range(2):
                    mm = ps.tile([128, TQ], F32, tag="mm", bufs=2, name="mm_v")
                    for dc in range(C):
                        wt = wpool.tile([128, 128], BF16, tag="wqk", bufs=10, name="wt_v")
                        nc.sync.dma_start(out=wt[:], in_=wv[si, dc * 128:(dc + 1) * 128,
                                                           rc * 128:(rc + 1) * 128])
                        nc.tensor.matmul(mm[:], wt[:], src[:, dc, th * TQ:(th + 1) * TQ],
                                         start=(dc == 0), stop=(dc == C - 1))
                    vt = cpool.tile([128, TQ], BF16, tag="vt", bufs=3, name="vt")
                    nc.vector.tensor_copy(vt[:], mm[:])
                    # transpose 128x128 blocks: v^T[vcol-blk rc, kpos] -> v[kpos, vcol]
                    for blk in range(TQ // 128):
                        kc = th * 4 + blk            # kpos chunk
                        tp = ps.tile([128, 128], BF16, tag="tr", bufs=2, name="tp_v")
                        nc.tensor.transpose(tp[:], vt[:, blk * 128:(blk + 1) * 128],
                                            ident_sb[:])
                        for hh in range(2):          # two heads per 128 vcols
                            h_idx = rc * 2 + hh
                            nc.vector.tensor_copy(
                                v_sb[:, kc, h_idx, 0:DK],
                                tp[:, hh * DK:(hh + 1) * DK])

            # ---- attention per head ---------------------------------------
            for h in range(H):
                base = (h % 2) * DK
                rc = h // 2
                exps = []
                for kc in range(C):
                    sc = ps.tile([128, TQ], F32, tag="sc", bufs=2, name="sc")
                    nc.tensor.matmul(sc[:],
                                     kT[base:base + DK, rc, kc * 128:(kc + 1) * 128],
                                     qT[base:base + DK, rc, :],
                                     start=True, stop=True)
                    e = cpool.tile([128, TQ], BF16, tag="exp", bufs=8, name="e_sc")
                    nc.scalar.activation(e[:], sc[:], AF.Exp, bias=0.0, scale=1.0)
                    exps.append(e)
                for qc in range(TQ // 128):
                    ov = ps.tile([128, DK + 1], F32, tag="av", bufs=2, name="ov")
                    for kc in range(C):
                        nc.tensor.matmul(ov[:], exps[kc][:, qc * 128:(qc + 1) * 128],
                                         v_sb[:, kc, h, :],
                                         start=(kc == 0), stop=(kc == C - 1))
                    inv = cpool.tile([128, 1], F32, tag="inv", bufs=4, name="inv")
                    nc.vector.reciprocal(inv[:], ov[:, DK:DK + 1])
                    nc.vector.tensor_scalar_mul(
                        o_sb[:, qc, h * DK:(h + 1) * DK], ov[:, 0:DK], inv[:])

            # ---- transpose o to o^T ---------------------------------------
            for qc in range(TQ // 128):
                for oc in range(C):
                    tp = ps.tile([128, 128], BF16, tag="tr", bufs=2, name="tp_o")
                    nc.tensor.transpose(tp[:], o_sb[:, qc, oc * 128:(oc + 1) * 128],
                                        ident_sb[:])
                    nc.vector.tensor_copy(oT_sb[:, oc, qc * 128:(qc + 1) * 128], tp[:])

            # ---- output projection ----------------------------------------
            for mc in range(C):
                mm = ps.tile([128, TQ], F32, tag="mm", bufs=2, name="mm_o")
                for oc in range(C):
                    wt = wpool.tile([128, 128], BF16, tag="wqk", bufs=10, name="wt_o")
                    nc.sync.dma_start(out=wt[:], in_=wo[oc * 128:(oc + 1) * 128,
                                                       mc * 128:(mc + 1) * 128])
                    nc.tensor.matmul(mm[:], wt[:], oT_sb[:, oc, :],
                                     start=(oc == 0), stop=(oc == C - 1))
                nc.scalar.activation(outs[si][:, mc, :], mm[:], AF.Identity,
                                     bias=ob_sb[:, si, mc:mc + 1], scale=1.0)

        # ---- gating -------------------------------------------------------
        lg = ps.tile([S, TQ], F32, tag="mm", bufs=2, name="lg")
        n_mm = S * C
        i_mm = 0
        for si in range(S):
            for c in range(C):
                nc.tensor.matmul(lg[:], gw_sb[:, si, c, :], outs[si][:, c, :],
                                 start=(i_mm == 0), stop=(i_mm == n_mm - 1))
                i_mm += 1
        eg = pers.tile([S, TQ], BF16, name="eg")
        nc.scalar.activation(eg[:], lg[:], AF.Exp, bias=gb_sb[:], scale=1.0)
        sg = ps.tile([1, TQ], F32, tag="mm", bufs=2, name="sg")
        nc.tensor.matmul(sg[:], ones_sb[0:S, :], eg[:], start=True, stop=True)
        invg = pers.tile([1, TQ], F32, name="invg")
        nc.vector.reciprocal(invg[:], sg[:])
        egf = pers.tile([S, TQ], F32, name="egf")
        invg3 = pers.tile([S, TQ], F32, name="invg3")
        nc.gpsimd.partition_broadcast(invg3[:], invg[:])
        nc.vector.tensor_mul(egf[:], eg[:], invg3[:])
        gbc = [pers.tile([128, TQ], F32, name=f"gbc{i}") for i in range(S)]
        egrows = [pers.tile([1, TQ], F32, name=f"egrow{i}") for i in range(S)]
        for i in range(S):
            # partition_broadcast needs its source at partition 0
            if i == 0:
                nc.gpsimd.partition_broadcast(gbc[i][:], egf[0:1, :])
            else:
                nc.sync.dma_start(out=egrows[i][:], in_=egf[i:i + 1, :])
                nc.gpsimd.partition_broadcast(gbc[i][:], egrows[i][:])

        # ---- combine + residual ------------------------------------------
        for c in range(C):
            t0 = cpool.tile([128, TQ], F32, tag="cmb", bufs=2, name="t0")
            t1 = cpool.tile([128, TQ], F32, tag="cmb", bufs=2, name="t1")
            nc.vector.tensor_mul(t0[:], outs[0][:, c, :], gbc[0][:])
            nc.vector.tensor_mul(t1[:], outs[1][:, c, :], gbc[1][:])
            nc.vector.tensor_add(t0[:], t0[:], t1[:])
            nc.vector.tensor_mul(t1[:], outs[2][:, c, :], gbc[2][:])
            nc.vector.tensor_add(t0[:], t0[:], t1[:])
            nc.vector.tensor_add(h_sb[:, c, :], t0[:], xT[:, c, 0:TQ])

        # ---- layernorm ----------------------------------------------------
        sum1 = ps.tile([1, TQ], F32, tag="mm", bufs=2, name="sum1")
        sum2 = ps.tile([1, TQ], F32, tag="mm", bufs=2, name="sum2")
        for c in range(C):
            hb = cpool.tile([128, TQ], BF16, tag="hb", bufs=2, name="hb")
            nc.scalar.copy(hb[:], h_sb[:, c, :])
            nc.tensor.matmul(sum1[:], ones_sb[:], hb[:],
                             start=(c == 0), stop=(c == C - 1))
        for c in range(C):
            hq = cpool.tile([128, TQ], BF16, tag="hb", bufs=2, name="hq")
            nc.scalar.square(hq[:], h_sb[:, c, :])
            nc.tensor.matmul(sum2[:], ones_sb[:], hq[:],
                             start=(c == 0), stop=(c == C - 1))
        mu = pers.tile([1, TQ], F32, name="mu")
        nc.scalar.mul(mu[:], sum1[:], 1.0 / D)
        m2 = pers.tile([1, TQ], F32, name="m2")
        nc.scalar.mul(m2[:], sum2[:], 1.0 / D)
        var = pers.tile([1, TQ], F32, name="var")
        nc.vector.tensor_mul(var[:], mu[:], mu[:])
        nc.vector.tensor_sub(var[:], m2[:], var[:])
        eps_sb = pers.tile([1, 1], F32, name="eps_sb")
        nc.vector.memset(eps_sb[:], LN_EPS)
        std = pers.tile([1, TQ], F32, name="std")
        nc.scalar.activation(std[:], var[:], AF.Sqrt, bias=eps_sb[:], scale=1.0)
        rstd = pers.tile([1, TQ], F32, name="rstd")
        nc.vector.reciprocal(rstd[:], std[:])
        mub = pers.tile([128, TQ], F32, name="mub")
        nc.gpsimd.partition_broadcast(mub[:], mu[:])
        rstdb = pers.tile([128, TQ], F32, name="rstdb")
        nc.gpsimd.partition_broadcast(rstdb[:], rstd[:])
        for c in range(C):
            dn = cpool.tile([128, TQ], F32, tag="cmb", bufs=2, name="dn")
            nc.vector.tensor_sub(dn[:], h_sb[:, c, :], mub[:])
            nc.vector.tensor_mul(dn[:], dn[:], rstdb[:])
            yc = cpool.tile([128, TQ], F32, tag="yc", bufs=2, name="yc")
            nc.scalar.activation(yc[:], dn[:], AF.Identity,
                                 bias=lnb_sb[:, c:c + 1], scale=lng_sb[:, c:c + 1])
            nc.sync.dma_start(out=yt.rearrange("(c p) t -> p c t", p=128)[:, c, :],
                              in_=yc[:])


def _window_sums(x, scale):
    """Unscaled zero-padded sliding window sums along axis 0. x: [T, D]."""
    p = scale // 2
    xp = np.pad(x.astype(np.float64), ((p, p), (0, 0)))
    cs = np.pad(np.cumsum(xp, axis=0), ((1, 0), (0, 0)))
    return cs[scale:] - cs[:-scale]


def _prep_inputs(x, qkv_w, qkv_b, out_w, out_b, gate_w, gate_b, ln_g, ln_b):
    bf = ml_dtypes.bfloat16
    wqk = np.empty((S, D, 2 * D), dtype=bf)
    wv = np.empty((S, D, D), dtype=bf)
    bq = np.empty((S, D), dtype=np.float32)
    ob = np.empty((S, D), dtype=np.float32)
    for i, scale in enumerate(SCALES):
        wqk[i, :, :D] = (qkv_w[i][:, :D] / (8.0 * scale)).astype(bf)
        wqk[i, :, D:] = (qkv_w[i][:, D:2 * D] / scale).astype(bf)
        wv[i] = (qkv_w[i][:, 2 * D:] / scale).astype(bf)
        bq[i] = qkv_b[i][:D] / 8.0
        ob[i] = out_b + qkv_b[i][2 * D:] @ out_w
    base = {
        "wqk": wqk, "wv": wv, "bq": bq,
        "wo": out_w.astype(bf), "ob": ob,
        "gw": np.ascontiguousarray(gate_w.reshape(S, D, S)).astype(bf),
        "gb": gate_b.astype(np.float32),
        "lng": ln_g.astype(np.float32), "lnb": ln_b.astype(np.float32),
        "ident": np.eye(128, dtype=bf),
    }
    in_maps = []
    for c in range(N_CORES):
        b, roll = c // 2, (c % 2) * TQ
        xr = np.roll(x[b], -roll, axis=0)                  # [T, D]
        corr = np.zeros((2, 8, C, 128), dtype=np.float32)
        if roll:
            prev = None
            for si, scale in enumerate((3, 5)):
                mine = _window_sums(xr, scale)             # device's zero-pad sums
                true = np.roll(_window_sums(x[b], scale), -roll, axis=0)
                delta = true - mine                        # [T, D]
                if prev is not None:
                    delta = delta - prev                   # scale-5 builds on corrected scale-3
                prev = (true - mine) if si == 0 else None
                nz = np.where(np.abs(delta).max(axis=1) > 1e-3)[0]
                assert set(nz.tolist()) <= set(FIX_COLS), nz
                for k, col in enumerate(FIX_COLS):
                    corr[si, k] = delta[col].reshape(C, 128)
        in_maps.append({**base,
                        "xt": np.ascontiguousarray(xr.T, dtype=np.float32),
                        "corr": corr})
    return in_maps


def kernel(x, qkv_w, qkv_b, out_w, out_b, gate_w, gate_b, ln_g, ln_b):
    if "nc" not in _cache:
        _cache["nc"] = _build()
    nc = _cache["nc"]
    in_maps = _prep_inputs(np.asarray(x, dtype=np.float32),
                           np.asarray(qkv_w, dtype=np.float32),
                           np.asarray(qkv_b, dtype=np.float32),
                           np.asarray(out_w, dtype=np.float32),
                           np.asarray(out_b, dtype=np.float32),
                           np.asarray(gate_w, dtype=np.float32),
                           np.asarray(gate_b, dtype=np.float32),
                           np.asarray(ln_g, dtype=np.float32),
                           np.asarray(ln_b, dtype=np.float32))
    res = run_bass_kernel_spmd(nc, in_maps, list(range(N_CORES)))
    out = np.empty((B, T, D), dtype=np.float32)
    for c in range(N_CORES):
        b, roll = c // 2, (c % 2) * TQ
        out[b, roll:roll + TQ, :] = res.results[c]["yt"].T
    return out


# revision 14
# speedup vs baseline: 1.2064x; 1.2064x over previous
"""Trainium2 Bass kernel for nn_EnhancedMultiHeadAttention (multi-scale MHA + gating + LN).

Sharding: 8 cores = 4 batches x 2 T-halves. Each core computes the full
attention stack for its (batch, query-half) on all 16 heads and 3 scales.
The query-half selection is made STATIC across the SPMD program by rolling
the time axis of x on the host for odd cores (attention is permutation-
invariant over key positions; avg-pool seam errors are fixed by small
host-computed correction vectors applied to 8 fixed columns).

All matmuls run in bf16 (fp32 PSUM accumulation). Softmax skips the max
subtraction (|logits| < 1 for these inputs by construction; verified).
"""
import numpy as np
import ml_dtypes

import concourse.bacc as bacc
import concourse.bass as bass
import concourse.mybir as mybir
from concourse import tile
from concourse.bass_utils import run_bass_kernel_spmd

F32 = mybir.dt.float32
BF16 = mybir.dt.bfloat16
AF = mybir.ActivationFunctionType

B, T, D = 4, 1024, 1024
H, DK = 16, 64
S = 3
SCALES = (1, 3, 5)
TQ = 512            # queries per core
C = D // 128        # 8 chunks of the model dim
N_CORES = 8
LN_EPS = 1e-5
# columns (rolled T index) that may need pooling seam corrections
FIX_COLS = (0, 1, 510, 511, 512, 513, 1022, 1023)

_cache = {}


def _build():
    nc = bacc.Bacc("TRN2", target_bir_lowering=False, debug=False,
                   enable_asserts=True, num_devices=N_CORES)

    xt = nc.dram_tensor("xt", [D, T], F32, kind="ExternalInput").ap()
    wqk = nc.dram_tensor("wqk", [S, D, 2 * D], BF16, kind="ExternalInput").ap()
    wv = nc.dram_tensor("wv", [S, D, D], BF16, kind="ExternalInput").ap()
    bq = nc.dram_tensor("bq", [S, D], F32, kind="ExternalInput").ap()
    wo = nc.dram_tensor("wo", [D, D], BF16, kind="ExternalInput").ap()
    ob = nc.dram_tensor("ob", [S, D], F32, kind="ExternalInput").ap()
    gw = nc.dram_tensor("gw", [S, D, S], BF16, kind="ExternalInput").ap()
    gb = nc.dram_tensor("gb", [S], F32, kind="ExternalInput").ap()
    lng = nc.dram_tensor("lng", [D], F32, kind="ExternalInput").ap()
    lnb = nc.dram_tensor("lnb", [D], F32, kind="ExternalInput").ap()
    ident = nc.dram_tensor("ident", [128, 128], BF16, kind="ExternalInput").ap()
    corr = nc.dram_tensor("corr", [2, 8, C, 128], F32, kind="ExternalInput").ap()
    yt = nc.dram_tensor("yt", [D, TQ], F32, kind="ExternalOutput").ap()

    with tile.TileContext(nc) as tc:
        _emit(tc, xt=xt, wqk=wqk, wv=wv, bq=bq, wo=wo, ob=ob, gw=gw, gb=gb,
              lng=lng, lnb=lnb, ident=ident, corr=corr, yt=yt)
    nc.compile()
    return nc


def _emit(tc, *, xt, wqk, wv, bq, wo, ob, gw, gb, lng, lnb, ident, corr, yt):
    nc = tc.nc
    pers_ctx = tc.tile_pool(name="pers", bufs=1)
    pers = pers_ctx.__enter__()

    # ---- persistent SBUF tensors -------------------------------------------
    xT = pers.tile([128, C, T], F32, name="xT")            # x^T  [D, T]
    A = pers.tile([128, C, T], BF16, name="A")             # pooled (unscaled) x^T
    qT = pers.tile([128, C, TQ], BF16, name="qT")          # q^T  [D, TQ]
    kT = pers.tile([128, C, T], BF16, name="kT")           # k^T  [D, T]
    v_sb = pers.tile([128, C, H, DK + 1], BF16, name="v_sb")   # v [T, D] (+ones col)
    oT_sb = pers.tile([128, C, TQ], BF16, name="oT_sb")    # o^T [D, TQ]
    outs = [pers.tile([128, C, TQ], BF16, name=f"out{i}") for i in range(S)]
    h_sb = pers.tile([128, C, TQ], F32, name="h_sb")       # residual+combined [D, TQ]

    ident_sb = pers.tile([128, 128], BF16, name="ident_sb")
    nc.sync.dma_start(out=ident_sb[:], in_=ident[:])
    bq_sb = pers.tile([128, S, C], F32, name="bq_sb")
    nc.sync.dma_start(out=bq_sb[:], in_=bq.rearrange("s (c p) -> p s c", p=128))
    ob_sb = pers.tile([128, S, C], F32, name="ob_sb")
    nc.sync.dma_start(out=ob_sb[:], in_=ob.rearrange("s (c p) -> p s c", p=128))
    gw_sb = pers.tile([128, S, C, S], BF16, name="gw_sb")
    nc.sync.dma_start(out=gw_sb[:], in_=gw.rearrange("s (c p) t -> p s c t", p=128))
    gb_sb = pers.tile([S, 1], F32, name="gb_sb")
    nc.sync.dma_start(out=gb_sb[:], in_=gb[:, None])
    lng_sb = pers.tile([128, C], F32, name="lng_sb")
    nc.sync.dma_start(out=lng_sb[:], in_=lng.rearrange("(c p) -> p c", p=128))
    lnb_sb = pers.tile([128, C], F32, name="lnb_sb")
    nc.sync.dma_start(out=lnb_sb[:], in_=lnb.rearrange("(c p) -> p c", p=128))
    corr_sb = pers.tile([128, 2, 8, C], F32, name="corr_sb")
    nc.sync.dma_start(out=corr_sb[:], in_=corr.rearrange("s k c p -> p s k c", p=128))

    nc.sync.dma_start(out=xT[:], in_=xt.rearrange("(c p) t -> p c t", p=128))
    nc.vector.memset(v_sb[:, :, :, DK:DK + 1], 1.0)      # ones col for denominator
    ones_sb = pers.tile([128, 1], BF16, name="ones_sb")
    nc.vector.memset(ones_sb[:], 1.0)
    s_all = pers.tile([H, TQ], F32, name="s_all")
    inv_all = pers.tile([H, TQ], F32, name="inv_all")

    with tc.tile_pool(name="wpool", bufs=2) as wpool, \
         tc.tile_pool(name="cpool", bufs=2) as cpool, \
         tc.tile_pool(name="ps", bufs=2, space="PSUM") as ps:

        for si, scale in enumerate(SCALES):
            # ---- pooling: A = unscaled window sum of xT along T ------------
            if scale == 3:
                nc.vector.tensor_add(A[:, :, 1:T], xT[:, :, 1:T], xT[:, :, 0:T - 1])
                nc.vector.tensor_copy(A[:, :, 0:1], xT[:, :, 0:1])
                nc.vector.tensor_add(A[:, :, 0:T - 1], A[:, :, 0:T - 1], xT[:, :, 1:T])
            elif scale == 5:
                nc.vector.tensor_add(A[:, :, 2:T], A[:, :, 2:T], xT[:, :, 0:T - 2])
                nc.vector.tensor_add(A[:, :, 0:T - 2], A[:, :, 0:T - 2], xT[:, :, 2:T])
            if scale != 1:
                for k in range(8):
                    col = FIX_COLS[k]
                    nc.vector.tensor_add(A[:, :, col], A[:, :, col],
                                         corr_sb[:, si - 1, k, :])
            src = xT if scale == 1 else A
            if scale == 1:
                # matmuls need bf16; make a bf16 copy of xT into A
                for c in range(C):
                    nc.vector.tensor_copy(A[:, c, :], xT[:, c, :])
                src = A

            # ---- q^T / k^T projections ------------------------------------
            # q^T rows 0:1024 = wqk cols 0:1024; k^T = wqk cols 1024:2048
            wqk_r = wqk[si].rearrange("(dc p) n -> p dc n", p=128)
            wv_r = wv[si].rearrange("(dc p) n -> p dc n", p=128)
            for rc in range(C):          # q row-chunks
                wt = wpool.tile([128, C, 128], BF16, tag="wqk", bufs=2, name="wt_q")
                nc.sync.dma_start(out=wt[:], in_=wqk_r[:, :, rc * 128:(rc + 1) * 128])
                mm = ps.tile([128, TQ], F32, tag="mm", bufs=2, name="mm_q")
                for dc in range(C):
                    nc.tensor.matmul(mm[:], wt[:, dc, :], src[:, dc, 0:TQ],
                                     start=(dc == 0), stop=(dc == C - 1))
                nc.scalar.activation(qT[:, rc, :], mm[:], AF.Identity,
                                     bias=bq_sb[:, si, rc:rc + 1], scale=1.0)
            for rc in range(C):          # k row-chunks
                wt = wpool.tile([128, C, 128], BF16, tag="wqk", bufs=2, name="wt_k")
                nc.sync.dma_start(out=wt[:],
                                  in_=wqk_r[:, :, D + rc * 128:D + (rc + 1) * 128])
                for th in range(2):
                    mm = ps.tile([128, TQ], F32, tag="mm", bufs=2, name="mm_k")
                    for dc in range(C):
                        nc.tensor.matmul(mm[:], wt[:, dc, :], src[:, dc, th * TQ:(th + 1) * TQ],
                                         start=(dc == 0), stop=(dc == C - 1))
                    nc.vector.tensor_copy(kT[:, rc, th * TQ:(th + 1) * TQ], mm[:])

            # ---- v^T projection + transpose to v [T, D] --------------------
            for rc in range(C):          # v^T row-chunks (v columns)
                wt = wpool.tile([128, C, 128], BF16, tag="wqk", bufs=2, name="wt_v")
                nc.sync.dma_start(out=wt[:], in_=wv_r[:, :, rc * 128:(rc + 1) * 128])
                for th in range(2):
                    mm = ps.tile([128, TQ], F32, tag="mm", bufs=2, name="mm_v")
                    for dc in range(C):
                        nc.tensor.matmul(mm[:], wt[:, dc, :], src[:, dc, th * TQ:(th + 1) * TQ],
                                         start=(dc == 0), stop=(dc == C - 1))
                    vt = cpool.tile([128, TQ], BF16, tag="vt", bufs=2, name="vt")
                    nc.vector.tensor_copy(vt[:], mm[:])
                    # transpose 128x128 blocks: v^T[vcol-blk rc, kpos] -> v[kpos, vcol]
                    for blk in range(TQ // 128):
                        kc = th * 4 + blk            # kpos chunk
                        tp = ps.tile([128, 128], BF16, tag="tr", bufs=2, name="tp_v")
                        nc.tensor.transpose(tp[:], vt[:, blk * 128:(blk + 1) * 128],
                                            ident_sb[:])
                        for hh in range(2):          # two heads per 128 vcols
                            h_idx = rc * 2 + hh
                            nc.vector.tensor_copy(
                                v_sb[:, kc, h_idx, 0:DK],
                                tp[:, hh * DK:(hh + 1) * DK])

            # ---- attention per head (o^T form, N=512 AV matmuls) ----------
            for h in range(H):
                base = (h % 2) * DK
                rc = h // 2
                exps = []
                for kc in range(C):
                    sc = ps.tile([128, TQ], F32, tag="sc", bufs=2, name="sc")
                    nc.tensor.matmul(sc[:],
                                     kT[base:base + DK, rc, kc * 128:(kc + 1) * 128],
                                     qT[base:base + DK, rc, :],
                                     start=True, stop=True)
                    e = cpool.tile([128, TQ], BF16, tag="exp", bufs=4, name="e_sc")
                    nc.scalar.activation(e[:], sc[:], AF.Exp, bias=0.0, scale=1.0)
                    exps.append(e)
                # o^T_h accumulated over kpos chunks; row DK = softmax denominator
                ov = ps.tile([DK + 1, TQ], F32, tag="av", bufs=2, name="ov")
                for kc in range(C):
                    nc.tensor.matmul(ov[:], v_sb[:, kc, h, :], exps[kc][:],
                                     start=(kc == 0), stop=(kc == C - 1))
                # unnormalized o^T -> oT_sb (odd heads via partition-moving DMA)
                if h % 2 == 0:
                    nc.vector.tensor_copy(oT_sb[0:DK, rc, :], ov[0:DK, :])
                else:
                    otmp = cpool.tile([DK, TQ], BF16, tag="otmp", bufs=1, name="otmp")
                    nc.vector.tensor_copy(otmp[:], ov[0:DK, :])
                    nc.gpsimd.dma_start(out=oT_sb[DK:128, rc, :], in_=otmp[:])
                # collect denominator row at partition h of s_all
                srow = cpool.tile([DK + 1, TQ], F32, tag="srow", bufs=1, name="srow")
                nc.vector.tensor_copy(srow[DK:DK + 1, :], ov[DK:DK + 1, :])
                nc.gpsimd.dma_start(out=s_all[h:h + 1, :], in_=srow[DK:DK + 1, :])
            # batch reciprocal + per-head broadcast, then normalize in place
            nc.vector.reciprocal(inv_all[:], s_all[:])
            for h in range(H):
                base = (h % 2) * DK
                rc = h // 2
                invRow = cpool.tile([1, TQ], F32, tag="invRow", bufs=2, name="invRow")
                nc.gpsimd.dma_start(out=invRow[:], in_=inv_all[h:h + 1, :])
                binv = cpool.tile([128, TQ], F32, tag="binv", bufs=2, name="binv")
                nc.gpsimd.partition_broadcast(binv[:], invRow[:])
                nc.vector.tensor_mul(oT_sb[base:base + DK, rc, :],
                                     oT_sb[base:base + DK, rc, :],
                                     binv[base:base + DK, :])

            # ---- output projection ----------------------------------------
            wo_r = wo.rearrange("(oc p) n -> p oc n", p=128)
            for mc in range(C):
                wt = wpool.tile([128, C, 128], BF16, tag="wqk", bufs=2, name="wt_o")
                nc.sync.dma_start(out=wt[:], in_=wo_r[:, :, mc * 128:(mc + 1) * 128])
                mm = ps.tile([128, TQ], F32, tag="mm", bufs=2, name="mm_o")
                for oc in range(C):
                    nc.tensor.matmul(mm[:], wt[:, oc, :], oT_sb[:, oc, :],
                                     start=(oc == 0), stop=(oc == C - 1))
                nc.scalar.activation(outs[si][:, mc, :], mm[:], AF.Identity,
                                     bias=ob_sb[:, si, mc:mc + 1], scale=1.0)

        # ---- gating -------------------------------------------------------
        lg = ps.tile([S, TQ], F32, tag="mm", bufs=2, name="lg")
        n_mm = S * C
        i_mm = 0
        for si in range(S):
            for c in range(C):
                nc.tensor.matmul(lg[:], gw_sb[:, si, c, :], outs[si][:, c, :],
                                 start=(i_mm == 0), stop=(i_mm == n_mm - 1))
                i_mm += 1
        eg = pers.tile([S, TQ], BF16, name="eg")
        nc.scalar.activation(eg[:], lg[:], AF.Exp, bias=gb_sb[:], scale=1.0)
        sg = ps.tile([1, TQ], F32, tag="mm", bufs=2, name="sg")
        nc.tensor.matmul(sg[:], ones_sb[0:S, :], eg[:], start=True, stop=True)
        invg = pers.tile([1, TQ], F32, name="invg")
        nc.vector.reciprocal(invg[:], sg[:])
        egf = pers.tile([S, TQ], F32, name="egf")
        invg3 = pers.tile([S, TQ], F32, name="invg3")
        nc.gpsimd.partition_broadcast(invg3[:], invg[:])
        nc.vector.tensor_mul(egf[:], eg[:], invg3[:])
        gbc = [pers.tile([128, TQ], F32, name=f"gbc{i}") for i in range(S)]
        egrows = {i: pers.tile([1, TQ], F32, name=f"egrow{i}") for i in (1, 2)}
        for i in range(S):
            # partition_broadcast needs its source at partition 0
            if i == 0:
                nc.gpsimd.partition_broadcast(gbc[i][:], egf[0:1, :])
            else:
                nc.sync.dma_start(out=egrows[i][:], in_=egf[i:i + 1, :])
                nc.gpsimd.partition_broadcast(gbc[i][:], egrows[i][:])

        # ---- combine + residual ------------------------------------------
        for c in range(C):
            t0 = cpool.tile([128, TQ], F32, tag="cmb", bufs=2, name="t0")
            t1 = cpool.tile([128, TQ], F32, tag="cmb", bufs=2, name="t1")
            nc.vector.tensor_mul(t0[:], outs[0][:, c, :], gbc[0][:])
            nc.vector.tensor_mul(t1[:], outs[1][:, c, :], gbc[1][:])
            nc.vector.tensor_add(t0[:], t0[:], t1[:])
            nc.vector.tensor_mul(t1[:], outs[2][:, c, :], gbc[2][:])
            nc.vector.tensor_add(t0[:], t0[:], t1[:])
            nc.vector.tensor_add(h_sb[:, c, :], t0[:], xT[:, c, 0:TQ])

        # ---- layernorm ----------------------------------------------------
        sum1 = ps.tile([1, TQ], F32, tag="mm", bufs=2, name="sum1")
        sum2 = ps.tile([1, TQ], F32, tag="mm", bufs=2, name="sum2")
        for c in range(C):
            hb = cpool.tile([128, TQ], BF16, tag="hb", bufs=2, name="hb")
            nc.scalar.copy(hb[:], h_sb[:, c, :])
            nc.tensor.matmul(sum1[:], ones_sb[:], hb[:],
                             start=(c == 0), stop=(c == C - 1))
        for c in range(C):
            hq = cpool.tile([128, TQ], BF16, tag="hb", bufs=2, name="hq")
            nc.scalar.square(hq[:], h_sb[:, c, :])
            nc.tensor.matmul(sum2[:], ones_sb[:], hq[:],
                             start=(c == 0), stop=(c == C - 1))
        mu = pers.tile([1, TQ], F32, name="mu")
        nc.scalar.mul(mu[:], sum1[:], 1.0 / D)
        m2 = pers.tile([1, TQ], F32, name="m2")
        nc.scalar.mul(m2[:], sum2[:], 1.0 / D)
        var = pers.tile([1, TQ], F32, name="var")
        nc.vector.tensor_mul(var[:], mu[:], mu[:])
        nc.vector.tensor_sub(var[:], m2[:], var[:])
        eps_sb = pers.tile([1, 1], F32, name="eps_sb")
        nc.vector.memset(eps_sb[:], LN_EPS)
        std = pers.tile([1, TQ], F32, name="std")
        nc.scalar.activation(std[:], var[:], AF.Sqrt, bias=eps_sb[:], scale=1.0)
        rstd = pers.tile([1, TQ], F32, name="rstd")
        nc.vector.reciprocal(rstd[:], std[:])
        mub = pers.tile([128, TQ], F32, name="mub")
        nc.gpsimd.partition_broadcast(mub[:], mu[:])
        rstdb = pers.tile([128, TQ], F32, name="rstdb")
        nc.gpsimd.partition_broadcast(rstdb[:], rstd[:])
        for c in range(C):
            dn = cpool.tile([128, TQ], F32, tag="cmb", bufs=2, name="dn")
            nc.vector.tensor_sub(dn[:], h_sb[:, c, :], mub[:])
            nc.vector.tensor_mul(dn[:], dn[:], rstdb[:])
            yc = cpool.tile([128, TQ], F32, tag="yc", bufs=2, name="yc")
            nc.scalar.activation(yc[:], dn[:], AF.Identity,
                                 bias=lnb_sb[:, c:c + 1], scale=lng_sb[:, c:c + 1])
            nc.sync.dma_start(out=yt.rearrange("(c p) t -> p c t", p=128)[:, c, :],
                              in_=yc[:])


def _window_sums(x, scale):
    """Unscaled zero-padded sliding window sums along axis 0. x: [T, D]."""
    p = scale // 2
    xp = np.pad(x.astype(np.float64), ((p, p), (0, 0)))
    cs = np.pad(np.cumsum(xp, axis=0), ((1, 0), (0, 0)))
    return cs[scale:] - cs[:-scale]


def _prep_inputs(x, qkv_w, qkv_b, out_w, out_b, gate_w, gate_b, ln_g, ln_b):
    bf = ml_dtypes.bfloat16
    wqk = np.empty((S, D, 2 * D), dtype=bf)
    wv = np.empty((S, D, D), dtype=bf)
    bq = np.empty((S, D), dtype=np.float32)
    ob = np.empty((S, D), dtype=np.float32)
    for i, scale in enumerate(SCALES):
        wqk[i, :, :D] = (qkv_w[i][:, :D] / (8.0 * scale)).astype(bf)
        wqk[i, :, D:] = (qkv_w[i][:, D:2 * D] / scale).astype(bf)
        wv[i] = (qkv_w[i][:, 2 * D:] / scale).astype(bf)
        bq[i] = qkv_b[i][:D] / 8.0
        ob[i] = out_b + qkv_b[i][2 * D:] @ out_w
    base = {
        "wqk": wqk, "wv": wv, "bq": bq,
        "wo": out_w.astype(bf), "ob": ob,
        "gw": np.ascontiguousarray(gate_w.reshape(S, D, S)).astype(bf),
        "gb": gate_b.astype(np.float32),
        "lng": ln_g.astype(np.float32), "lnb": ln_b.astype(np.float32),
        "ident": np.eye(128, dtype=bf),
    }
    in_maps = []
    for c in range(N_CORES):
        b, roll = c // 2, (c % 2) * TQ
        xr = np.roll(x[b], -roll, axis=0)                  # [T, D]
        corr = np.zeros((2, 8, C, 128), dtype=np.float32)
        if roll:
            prev = None
            for si, scale in enumerate((3, 5)):
                mine = _window_sums(xr, scale)             # device's zero-pad sums
                true = np.roll(_window_sums(x[b], scale), -roll, axis=0)
                delta = true - mine                        # [T, D]
                if prev is not None:
                    delta = delta - prev                   # scale-5 builds on corrected scale-3
                prev = (true - mine) if si == 0 else None
                nz = np.where(np.abs(delta).max(axis=1) > 1e-3)[0]
                assert set(nz.tolist()) <= set(FIX_COLS), nz
                for k, col in enumerate(FIX_COLS):
                    corr[si, k] = delta[col].reshape(C, 128)
        in_maps.append({**base,
                        "xt": np.ascontiguousarray(xr.T, dtype=np.float32),
                        "corr": corr})
    return in_maps


def kernel(x, qkv_w, qkv_b, out_w, out_b, gate_w, gate_b, ln_g, ln_b):
    if "nc" not in _cache:
        _cache["nc"] = _build()
    nc = _cache["nc"]
    in_maps = _prep_inputs(np.asarray(x, dtype=np.float32),
                           np.asarray(qkv_w, dtype=np.float32),
                           np.asarray(qkv_b, dtype=np.float32),
                           np.asarray(out_w, dtype=np.float32),
                           np.asarray(out_b, dtype=np.float32),
                           np.asarray(gate_w, dtype=np.float32),
                           np.asarray(gate_b, dtype=np.float32),
                           np.asarray(ln_g, dtype=np.float32),
                           np.asarray(ln_b, dtype=np.float32))
    res = run_bass_kernel_spmd(nc, in_maps, list(range(N_CORES)))
    out = np.empty((B, T, D), dtype=np.float32)
    for c in range(N_CORES):
        b, roll = c // 2, (c % 2) * TQ
        out[b, roll:roll + TQ, :] = res.results[c]["yt"].T
    return out


# revision 18
# speedup vs baseline: 1.4773x; 1.2246x over previous
"""Trainium2 Bass kernel for nn_EnhancedMultiHeadAttention (multi-scale MHA + gating + LN).

Sharding: 8 cores = 4 batches x 2 T-halves. Each core computes the full
attention stack for its (batch, query-half) on all 16 heads and 3 scales.
The query-half selection is made STATIC across the SPMD program by rolling
the time axis of x on the host for odd cores (attention is permutation-
invariant over key positions; avg-pool seam errors are fixed by small
host-computed correction vectors applied to 8 fixed columns).

All matmuls run in bf16 (fp32 PSUM accumulation). Softmax skips the max
subtraction (|logits| < 1 for these inputs by construction; verified).
"""
import numpy as np
import ml_dtypes

import concourse.bacc as bacc
import concourse.bass as bass
import concourse.mybir as mybir
from concourse import tile
from concourse.bass_utils import run_bass_kernel_spmd

F32 = mybir.dt.float32
BF16 = mybir.dt.bfloat16
AF = mybir.ActivationFunctionType

B, T, D = 4, 1024, 1024
H, DK = 16, 64
S = 3
SCALES = (1, 3, 5)
TQ = 512            # queries per core
C = D // 128        # 8 chunks of the model dim
N_CORES = 8
LN_EPS = 1e-5
# columns (rolled T index) that may need pooling seam corrections
FIX_COLS = (0, 1, 510, 511, 512, 513, 1022, 1023)

_cache = {}


def _build():
    nc = bacc.Bacc("TRN2", target_bir_lowering=False, debug=False,
                   enable_asserts=True, num_devices=N_CORES)

    xt = nc.dram_tensor("xt", [D, T], F32, kind="ExternalInput").ap()
    wqk = nc.dram_tensor("wqk", [S, D, 2 * D], BF16, kind="ExternalInput").ap()
    wv = nc.dram_tensor("wv", [S, D, D], BF16, kind="ExternalInput").ap()
    bq = nc.dram_tensor("bq", [S, D], F32, kind="ExternalInput").ap()
    wo = nc.dram_tensor("wo", [D, D], BF16, kind="ExternalInput").ap()
    ob = nc.dram_tensor("ob", [S, D], F32, kind="ExternalInput").ap()
    gw = nc.dram_tensor("gw", [S, D, S], BF16, kind="ExternalInput").ap()
    gb = nc.dram_tensor("gb", [S], F32, kind="ExternalInput").ap()
    lng = nc.dram_tensor("lng", [D], F32, kind="ExternalInput").ap()
    lnb = nc.dram_tensor("lnb", [D], F32, kind="ExternalInput").ap()
    ident = nc.dram_tensor("ident", [128, 128], BF16, kind="ExternalInput").ap()
    corr = nc.dram_tensor("corr", [2, 8, C, 128], F32, kind="ExternalInput").ap()
    yt = nc.dram_tensor("yt", [D, TQ], F32, kind="ExternalOutput").ap()

    with tile.TileContext(nc) as tc:
        _emit(tc, xt=xt, wqk=wqk, wv=wv, bq=bq, wo=wo, ob=ob, gw=gw, gb=gb,
              lng=lng, lnb=lnb, ident=ident, corr=corr, yt=yt)
    nc.compile()
    return nc


def _emit(tc, *, xt, wqk, wv, bq, wo, ob, gw, gb, lng, lnb, ident, corr, yt):
    nc = tc.nc
    pers_ctx = tc.tile_pool(name="pers", bufs=1)
    pers = pers_ctx.__enter__()

    # ---- persistent SBUF tensors -------------------------------------------
    xT = pers.tile([128, C, T], F32, name="xT")            # x^T  [D, T]
    A = pers.tile([128, C, T], BF16, name="A")             # pooled (unscaled) x^T
    qT = pers.tile([128, C, TQ], BF16, name="qT")          # q^T  [D, TQ]
    kT = pers.tile([128, C, T], BF16, name="kT")           # k^T  [D, T]
    v_sb = pers.tile([128, C, H, DK + 1], BF16, name="v_sb")   # v [T, D] (+ones col)
    oT_sb = pers.tile([128, C, TQ], BF16, name="oT_sb")    # o^T [D, TQ]
    outs = [pers.tile([128, C, TQ], BF16, name=f"out{i}") for i in range(S)]
    h_sb = pers.tile([128, C, TQ], F32, name="h_sb")       # residual+combined [D, TQ]

    ident_sb = pers.tile([128, 128], BF16, name="ident_sb")
    nc.sync.dma_start(out=ident_sb[:], in_=ident[:])
    bq_sb = pers.tile([128, S, C], F32, name="bq_sb")
    nc.sync.dma_start(out=bq_sb[:], in_=bq.rearrange("s (c p) -> p s c", p=128))
    ob_sb = pers.tile([128, S, C], F32, name="ob_sb")
    nc.sync.dma_start(out=ob_sb[:], in_=ob.rearrange("s (c p) -> p s c", p=128))
    gw_sb = pers.tile([128, S, C, S], BF16, name="gw_sb")
    nc.sync.dma_start(out=gw_sb[:], in_=gw.rearrange("s (c p) t -> p s c t", p=128))
    gb_sb = pers.tile([S, 1], F32, name="gb_sb")
    nc.sync.dma_start(out=gb_sb[:], in_=gb[:, None])
    lng_sb = pers.tile([128, C], F32, name="lng_sb")
    nc.sync.dma_start(out=lng_sb[:], in_=lng.rearrange("(c p) -> p c", p=128))
    lnb_sb = pers.tile([128, C], F32, name="lnb_sb")
    nc.sync.dma_start(out=lnb_sb[:], in_=lnb.rearrange("(c p) -> p c", p=128))
    corr_sb = pers.tile([128, 2, 8, C], F32, name="corr_sb")
    nc.sync.dma_start(out=corr_sb[:], in_=corr.rearrange("s k c p -> p s k c", p=128))

    nc.sync.dma_start(out=xT[:], in_=xt.rearrange("(c p) t -> p c t", p=128))
    nc.vector.memset(v_sb[:, :, :, DK:DK + 1], 1.0)      # ones col for denominator
    ones_sb = pers.tile([128, 1], BF16, name="ones_sb")
    nc.vector.memset(ones_sb[:], 1.0)
    s_all = pers.tile([H, TQ], F32, name="s_all")
    inv_all = pers.tile([H, TQ], F32, name="inv_all")
    lgp = pers.tile([S, S, TQ], BF16, name="lgp")           # per-scale gate logits

    wpool_ctx = tc.tile_pool(name="wpool", bufs=2)
    wpool = wpool_ctx.__enter__()
    cpool_ctx = tc.tile_pool(name="cpool", bufs=2)
    cpool = cpool_ctx.__enter__()
    ps_ctx = tc.tile_pool(name="ps", bufs=2, space="PSUM")
    ps = ps_ctx.__enter__()

    def pool(si):
        scale = SCALES[si]
        if scale == 1:
            # matmuls need bf16; A starts as a bf16 copy of xT
            for c in range(C):
                nc.vector.tensor_copy(A[:, c, :], xT[:, c, :])
            return
        if scale == 3:
            nc.vector.tensor_add(A[:, :, 1:T], xT[:, :, 1:T], xT[:, :, 0:T - 1])
            nc.vector.tensor_copy(A[:, :, 0:1], xT[:, :, 0:1])
            nc.vector.tensor_add(A[:, :, 0:T - 1], A[:, :, 0:T - 1], xT[:, :, 1:T])
        elif scale == 5:
            nc.vector.tensor_add(A[:, :, 2:T], A[:, :, 2:T], xT[:, :, 0:T - 2])
            nc.vector.tensor_add(A[:, :, 0:T - 2], A[:, :, 0:T - 2], xT[:, :, 2:T])
        for k in range(8):
            col = FIX_COLS[k]
            nc.vector.tensor_add(A[:, :, col], A[:, :, col],
                                 corr_sb[:, si - 1, k, :])

    def qkv(si):
        wqk_r = wqk[si].rearrange("(dc p) n -> p dc n", p=128)
        wv_r = wv[si].rearrange("(dc p) n -> p dc n", p=128)
        for rc in range(C):          # q row-chunks
            wt = wpool.tile([128, C, 128], BF16, tag="wqk", bufs=2, name="wt_q")
            nc.sync.dma_start(out=wt[:], in_=wqk_r[:, :, rc * 128:(rc + 1) * 128])
            mm = ps.tile([128, TQ], F32, tag="mm", bufs=2, name="mm_q")
            for dc in range(C):
                nc.tensor.matmul(mm[:], wt[:, dc, :], A[:, dc, 0:TQ],
                                 start=(dc == 0), stop=(dc == C - 1))
            nc.vector.tensor_scalar_add(qT[:, rc, :], mm[:], bq_sb[:, si, rc:rc + 1])
        for rc in range(C):          # k row-chunks
            wt = wpool.tile([128, C, 128], BF16, tag="wqk", bufs=2, name="wt_k")
            nc.sync.dma_start(out=wt[:],
                              in_=wqk_r[:, :, D + rc * 128:D + (rc + 1) * 128])
            for th in range(2):
                mm = ps.tile([128, TQ], F32, tag="mm", bufs=2, name="mm_k")
                for dc in range(C):
                    nc.tensor.matmul(mm[:], wt[:, dc, :], A[:, dc, th * TQ:(th + 1) * TQ],
                                     start=(dc == 0), stop=(dc == C - 1))
                nc.vector.tensor_copy(kT[:, rc, th * TQ:(th + 1) * TQ], mm[:])
        for rc in range(C):          # v^T row-chunks, then transpose to v [T, D]
            wt = wpool.tile([128, C, 128], BF16, tag="wqk", bufs=2, name="wt_v")
            nc.sync.dma_start(out=wt[:], in_=wv_r[:, :, rc * 128:(rc + 1) * 128])
            for th in range(2):
                mm = ps.tile([128, TQ], F32, tag="mm", bufs=2, name="mm_v")
                for dc in range(C):
                    nc.tensor.matmul(mm[:], wt[:, dc, :], A[:, dc, th * TQ:(th + 1) * TQ],
                                     start=(dc == 0), stop=(dc == C - 1))
                vt = cpool.tile([128, TQ], BF16, tag="vt", bufs=2, name="vt")
                nc.vector.tensor_copy(vt[:], mm[:])
                for blk in range(TQ // 128):
                    kc = th * 4 + blk            # kpos chunk
                    tp = ps.tile([128, 128], BF16, tag="tr", bufs=2, name="tp_v")
                    nc.tensor.transpose(tp[:], vt[:, blk * 128:(blk + 1) * 128],
                                        ident_sb[:])
                    for hh in range(2):          # two heads per 128 vcols
                        nc.vector.tensor_copy(v_sb[:, kc, rc * 2 + hh, 0:DK],
                                              tp[:, hh * DK:(hh + 1) * DK])

    def att(si):
        for h in range(H):
            base = (h % 2) * DK
            rc = h // 2
            exps = []
            for kc in range(C):
                sc = ps.tile([128, TQ], F32, tag="sc", bufs=2, name="sc")
                nc.tensor.matmul(sc[:],
                                 kT[base:base + DK, rc, kc * 128:(kc + 1) * 128],
                                 qT[base:base + DK, rc, :],
                                 start=True, stop=True)
                e = cpool.tile([128, TQ], BF16, tag="exp", bufs=4, name="e_sc")
                nc.scalar.activation(e[:], sc[:], AF.Exp, bias=0.0, scale=1.0)
                exps.append(e)
            # o^T_h accumulated over kpos chunks; row DK = softmax denominator
            ov = ps.tile([DK + 1, TQ], F32, tag="av", bufs=2, name="ov")
            for kc in range(C):
                nc.tensor.matmul(ov[:], v_sb[:, kc, h, :], exps[kc][:],
                                 start=(kc == 0), stop=(kc == C - 1))
            # unnormalized o^T -> oT_sb (odd heads via partition-moving DMA)
            if h % 2 == 0:
                nc.vector.tensor_copy(oT_sb[0:DK, rc, :], ov[0:DK, :])
            else:
                otmp = cpool.tile([DK, TQ], BF16, tag="otmp", bufs=1, name="otmp")
                nc.vector.tensor_copy(otmp[:], ov[0:DK, :])
                nc.gpsimd.dma_start(out=oT_sb[DK:128, rc, :], in_=otmp[:])
            # collect denominator row at partition h of s_all
            srow = cpool.tile([DK + 1, TQ], F32, tag="srow", bufs=1, name="srow")
            nc.vector.tensor_copy(srow[DK:DK + 1, :], ov[DK:DK + 1, :])
            nc.gpsimd.dma_start(out=s_all[h:h + 1, :], in_=srow[DK:DK + 1, :])

    def norm(si):
        nc.vector.reciprocal(inv_all[:], s_all[:])
        for h in range(H):
            base = (h % 2) * DK
            rc = h // 2
            invRow = cpool.tile([1, TQ], F32, tag="invRow", bufs=2, name="invRow")
            nc.gpsimd.dma_start(out=invRow[:], in_=inv_all[h:h + 1, :])
            binv = cpool.tile([128, TQ], F32, tag="binv", bufs=2, name="binv")
            nc.gpsimd.partition_broadcast(binv[:], invRow[:])
            nc.vector.tensor_mul(oT_sb[base:base + DK, rc, :],
                                 oT_sb[base:base + DK, rc, :],
                                 binv[base:base + DK, :])

    def outproj(si):
        wo_r = wo.rearrange("(oc p) n -> p oc n", p=128)
        for mc in range(C):
            wt = wpool.tile([128, C, 128], BF16, tag="wqk", bufs=2, name="wt_o")
            nc.sync.dma_start(out=wt[:], in_=wo_r[:, :, mc * 128:(mc + 1) * 128])
            mm = ps.tile([128, TQ], F32, tag="mm", bufs=2, name="mm_o")
            for oc in range(C):
                nc.tensor.matmul(mm[:], wt[:, oc, :], oT_sb[:, oc, :],
                                 start=(oc == 0), stop=(oc == C - 1))
            nc.vector.tensor_scalar_add(outs[si][:, mc, :], mm[:],
                                        ob_sb[:, si, mc:mc + 1])

    def logits(si):
        lg = ps.tile([S, TQ], F32, tag="mm", bufs=2, name="lg")
        for c in range(C):
            nc.tensor.matmul(lg[:], gw_sb[:, si, c, :], outs[si][:, c, :],
                             start=(c == 0), stop=(c == C - 1))
        nc.vector.tensor_copy(lgp[:, si, :], lg[:])

    # interleaved emission: the next scale's projections fill the PE while the
    # current scale's normalize/gating runs on DVE/GpSimd
    pool(0)
    qkv(0)
    pool(1)
    att(0)
    norm(0)
    qkv(1)
    outproj(0)
    logits(0)
    pool(2)
    att(1)
    norm(1)
    qkv(2)
    outproj(1)
    logits(1)
    att(2)
    norm(2)
    outproj(2)
    logits(2)

    # ---- gating -----------------------------------------------------------
    lgs = pers.tile([S, TQ], F32, name="lgs")
    nc.vector.tensor_add(lgs[:], lgp[:, 0, :], lgp[:, 1, :])
    nc.vector.tensor_add(lgs[:], lgs[:], lgp[:, 2, :])
    eg = pers.tile([S, TQ], BF16, name="eg")
    nc.scalar.activation(eg[:], lgs[:], AF.Exp, bias=gb_sb[:], scale=1.0)
    sg = ps.tile([1, TQ], F32, tag="mm", bufs=2, name="sg")
    nc.tensor.matmul(sg[:], ones_sb[0:S, :], eg[:], start=True, stop=True)
    invg = pers.tile([1, TQ], F32, name="invg")
    nc.vector.reciprocal(invg[:], sg[:])
    invgB = pers.tile([128, TQ], F32, name="invgB")
    nc.gpsimd.partition_broadcast(invgB[:], invg[:])
    gbc = [pers.tile([128, TQ], BF16, name=f"gbc{i}") for i in range(S)]
    egrows = {i: pers.tile([1, TQ], BF16, name=f"egrow{i}") for i in (1, 2)}
    for i in range(S):
        # partition_broadcast needs its source at partition 0
        if i == 0:
            nc.gpsimd.partition_broadcast(gbc[i][:], eg[0:1, :])
        else:
            nc.gpsimd.dma_start(out=egrows[i][:], in_=eg[i:i + 1, :])
            nc.gpsimd.partition_broadcast(gbc[i][:], egrows[i][:])

    # ---- combine + residual (gates normalized by a single 1/sum pass) -----
    for c in range(C):
        t0 = cpool.tile([128, TQ], F32, tag="cmb", bufs=2, name="t0")
        t1 = cpool.tile([128, TQ], F32, tag="cmb", bufs=2, name="t1")
        nc.vector.tensor_mul(t0[:], outs[0][:, c, :], gbc[0][:])
        nc.vector.tensor_mul(t1[:], outs[1][:, c, :], gbc[1][:])
        nc.vector.tensor_add(t0[:], t0[:], t1[:])
        nc.vector.tensor_mul(t1[:], outs[2][:, c, :], gbc[2][:])
        nc.vector.tensor_add(t0[:], t0[:], t1[:])
        nc.vector.tensor_mul(t0[:], t0[:], invgB[:])
        nc.vector.tensor_add(h_sb[:, c, :], t0[:], xT[:, c, 0:TQ])

    # ---- layernorm ---------------------------------------------------------
    sum1 = ps.tile([1, TQ], F32, tag="mm", bufs=2, name="sum1")
    sum2 = ps.tile([1, TQ], F32, tag="mm", bufs=2, name="sum2")
    for c in range(C):
        hb = cpool.tile([128, TQ], BF16, tag="hb", bufs=2, name="hb")
        nc.scalar.copy(hb[:], h_sb[:, c, :])
        nc.tensor.matmul(sum1[:], ones_sb[:], hb[:],
                         start=(c == 0), stop=(c == C - 1))
    for c in range(C):
        hq = cpool.tile([128, TQ], BF16, tag="hb", bufs=2, name="hq")
        nc.scalar.square(hq[:], h_sb[:, c, :])
        nc.tensor.matmul(sum2[:], ones_sb[:], hq[:],
                         start=(c == 0), stop=(c == C - 1))
    mu = pers.tile([1, TQ], F32, name="mu")
    nc.scalar.mul(mu[:], sum1[:], 1.0 / D)
    m2 = pers.tile([1, TQ], F32, name="m2")
    nc.scalar.mul(m2[:], sum2[:], 1.0 / D)
    var = pers.tile([1, TQ], F32, name="var")
    nc.vector.tensor_mul(var[:], mu[:], mu[:])
    nc.vector.tensor_sub(var[:], m2[:], var[:])
    eps_sb = pers.tile([1, 1], F32, name="eps_sb")
    nc.vector.memset(eps_sb[:], LN_EPS)
    std = pers.tile([1, TQ], F32, name="std")
    nc.scalar.activation(std[:], var[:], AF.Sqrt, bias=eps_sb[:], scale=1.0)
    rstd = pers.tile([1, TQ], F32, name="rstd")
    nc.vector.reciprocal(rstd[:], std[:])
    mub = pers.tile([128, TQ], F32, name="mub")
    nc.gpsimd.partition_broadcast(mub[:], mu[:])
    rstdb = pers.tile([128, TQ], F32, name="rstdb")
    nc.gpsimd.partition_broadcast(rstdb[:], rstd[:])
    for c in range(C):
        dn = cpool.tile([128, TQ], F32, tag="cmb", bufs=2, name="dn")
        nc.vector.tensor_sub(dn[:], h_sb[:, c, :], mub[:])
        nc.vector.tensor_mul(dn[:], dn[:], rstdb[:])
        yc = cpool.tile([128, TQ], F32, tag="yc", bufs=2, name="yc")
        nc.scalar.activation(yc[:], dn[:], AF.Identity,
                             bias=lnb_sb[:, c:c + 1], scale=lng_sb[:, c:c + 1])
        nc.sync.dma_start(out=yt.rearrange("(c p) t -> p c t", p=128)[:, c, :],
                          in_=yc[:])

    ps_ctx.__exit__(None, None, None)
    cpool_ctx.__exit__(None, None, None)
    wpool_ctx.__exit__(None, None, None)
    pers_ctx.__exit__(None, None, None)


def _window_sums(x, scale):
    """Unscaled zero-padded sliding window sums along axis 0. x: [T, D]."""
    p = scale // 2
    xp = np.pad(x.astype(np.float64), ((p, p), (0, 0)))
    cs = np.pad(np.cumsum(xp, axis=0), ((1, 0), (0, 0)))
    return cs[scale:] - cs[:-scale]


def _prep_inputs(x, qkv_w, qkv_b, out_w, out_b, gate_w, gate_b, ln_g, ln_b):
    bf = ml_dtypes.bfloat16
    wqk = np.empty((S, D, 2 * D), dtype=bf)
    wv = np.empty((S, D, D), dtype=bf)
    bq = np.empty((S, D), dtype=np.float32)
    ob = np.empty((S, D), dtype=np.float32)
    for i, scale in enumerate(SCALES):
        wqk[i, :, :D] = (qkv_w[i][:, :D] / (8.0 * scale)).astype(bf)
        wqk[i, :, D:] = (qkv_w[i][:, D:2 * D] / scale).astype(bf)
        wv[i] = (qkv_w[i][:, 2 * D:] / scale).astype(bf)
        bq[i] = qkv_b[i][:D] / 8.0
        ob[i] = out_b + qkv_b[i][2 * D:] @ out_w
    base = {
        "wqk": wqk, "wv": wv, "bq": bq,
        "wo": out_w.astype(bf), "ob": ob,
        "gw": np.ascontiguousarray(gate_w.reshape(S, D, S)).astype(bf),
        "gb": gate_b.astype(np.float32),
        "lng": ln_g.astype(np.float32), "lnb": ln_b.astype(np.float32),
        "ident": np.eye(128, dtype=bf),
    }
    in_maps = []
    for c in range(N_CORES):
        b, roll = c // 2, (c % 2) * TQ
        xr = np.roll(x[b], -roll, axis=0)                  # [T, D]
        corr = np.zeros((2, 8, C, 128), dtype=np.float32)
        if roll:
            prev = None
            for si, scale in enumerate((3, 5)):
                mine = _window_sums(xr, scale)             # device's zero-pad sums
                true = np.roll(_window_sums(x[b], scale), -roll, axis=0)
                delta = true - mine                        # [T, D]
                if prev is not None:
                    delta = delta - prev                   # scale-5 builds on corrected scale-3
                prev = (true - mine) if si == 0 else None
                nz = np.where(np.abs(delta).max(axis=1) > 1e-3)[0]
                assert set(nz.tolist()) <= set(FIX_COLS), nz
                for k, col in enumerate(FIX_COLS):
                    corr[si, k] = delta[col].reshape(C, 128)
        in_maps.append({**base,
                        "xt": np.ascontiguousarray(xr.T, dtype=np.float32),
                        "corr": corr})
    return in_maps


def kernel(x, qkv_w, qkv_b, out_w, out_b, gate_w, gate_b, ln_g, ln_b):
    if "nc" not in _cache:
        _cache["nc"] = _build()
    nc = _cache["nc"]
    in_maps = _prep_inputs(np.asarray(x, dtype=np.float32),
                           np.asarray(qkv_w, dtype=np.float32),
                           np.asarray(qkv_b, dtype=np.float32),
                           np.asarray(out_w, dtype=np.float32),
                           np.asarray(out_b, dtype=np.float32),
                           np.asarray(gate_w, dtype=np.float32),
                           np.asarray(gate_b, dtype=np.float32),
                           np.asarray(ln_g, dtype=np.float32),
                           np.asarray(ln_b, dtype=np.float32))
    res = run_bass_kernel_spmd(nc, in_maps, list(range(N_CORES)))
    out = np.empty((B, T, D), dtype=np.float32)
    for c in range(N_CORES):
        b, roll = c // 2, (c % 2) * TQ
        out[b, roll:roll + TQ, :] = res.results[c]["yt"].T
    return out
